# revision 20
# baseline (speedup 1.0000x reference)
# Trainium2 Bass kernel for nn_AxialAttention (8 NeuronCores, head/W-parallel).
#
# Sharding: the W axis (axis=2, the vmapped axis) is split into 8 contiguous
# slices of 32 columns, one per core. Every part of the computation (the four
# 1x1-conv GEMMs, the per-(head, w) axial attention, the embedding terms) is
# independent across w, so there are no collectives; the small weight matrices
# and embedding tables are replicated to every core.
#
# Per-core math for one w column (all heads):
#   qsT[x, (h c)] = query[:, :, w].T @ (Wq.T / 16)    (scale folded into Wq)
#   khT[x, (h c)] = key_[:, :, w].T @ Wk.T
#   vh [(h c), x] = Wv @ value[:, :, w]
#   logits_h[C, c] = khT_h.T @ qsT_h + q_emb.T @ qsT_h + k_emb.T @ khT_h
#   E = exp(logits)             (max-subtraction unnecessary: |logits| < ~2)
#   U_h = E_h.T @ [vh_h + ve | 1]          (ones column gives the softmax
#   attn_h = U_h[:, :256] / U_h[:, 256]     denominator for free)
#   out[:, :, w] = Wo @ attn
#
# Heads are packed even/odd into the two 64-partition halves so the per-head
# 64x64 logits matmuls and the 64-row attention matmuls run as concurrent
# PE row/column tiles (tile_position diagonal packing). All matmuls run in
# bf16 with fp32 PSUM accumulation (measured 3.3e-3 absmax-relative error);
# large PSUM->SBUF evacuations are split across the Scalar and Vector engines
# to halve PSUM-slot release latency.

import numpy as np

H = 8          # heads
QK = 64        # per-head qk/vo channels
C = 512        # io channels
X = 256        # spatial H (attention contraction axis)
W = 256        # spatial W (vmapped axis, sharded)
N_CORES = 8
WC = W // N_CORES   # w columns per core
PAIRS = WC // 2

_CACHE = {}


def _build_program():
    import concourse.mybir as mybir
    import concourse.tile as tile
    from concourse import bacc

    f32 = mybir.dt.float32
    bf16 = mybir.dt.bfloat16
    AF = mybir.ActivationFunctionType

    nc = bacc.Bacc("TRN2", target_bir_lowering=False, debug=False,
                   num_devices=N_CORES)

    qin = nc.dram_tensor("qin", [PAIRS, C, 2, X], bf16, kind="ExternalInput").ap()
    kin = nc.dram_tensor("kin", [PAIRS, C, 2, X], bf16, kind="ExternalInput").ap()
    vin = nc.dram_tensor("vin", [PAIRS, C, 2, X], bf16, kind="ExternalInput").ap()
    wqt = nc.dram_tensor("wqt", [C, C], bf16, kind="ExternalInput").ap()
    wkt = nc.dram_tensor("wkt", [C, C], bf16, kind="ExternalInput").ap()
    wvt = nc.dram_tensor("wvt", [C, C], bf16, kind="ExternalInput").ap()
    wot = nc.dram_tensor("wot", [C, C], bf16, kind="ExternalInput").ap()
    qe8 = nc.dram_tensor("qe8", [X, H * QK], bf16, kind="ExternalInput").ap()
    ke2 = nc.dram_tensor("ke2", [X, 2 * QK], bf16, kind="ExternalInput").ap()
    vet = nc.dram_tensor("vet", [QK, X], f32, kind="ExternalInput").ap()
    oned = nc.dram_tensor("oned", [128, 4], bf16, kind="ExternalInput").ap()
    out = nc.dram_tensor("out", [C, WC, X], f32, kind="ExternalOutput").ap()

    KT = C // 128   # 4 contraction tiles of the channel dim
    XT = X // 128   # 2 tiles of the spatial-x dim

    with tile.TileContext(nc) as tc:
        with (
            tc.tile_pool(name="consts", bufs=1) as consts,
            tc.tile_pool(name="inp", bufs=3) as inp,
            tc.tile_pool(name="qkt", bufs=2) as qkt,
            tc.tile_pool(name="mid", bufs=2) as mid,
            tc.tile_pool(name="small", bufs=8) as small,
            tc.tile_pool(name="psA", bufs=4, space="PSUM") as psA,
            tc.tile_pool(name="psVL", bufs=2, space="PSUM") as psVL,
            tc.tile_pool(name="psU", bufs=2, space="PSUM") as psU,
        ):
            def load_inputs(pair):
                q_t = inp.tile([128, KT, 2, X], bf16, tag="q_t")
                nc.sync.dma_start(
                    q_t[:], qin[pair].rearrange("(kt p) w x -> p kt (w x)", p=128))
                k_t = inp.tile([128, KT, 2, X], bf16, tag="k_t")
                nc.sync.dma_start(
                    k_t[:], kin[pair].rearrange("(kt p) w x -> p kt (w x)", p=128))
                v_t = inp.tile([128, KT, 2, X], bf16, tag="v_t")
                nc.sync.dma_start(
                    v_t[:], vin[pair].rearrange("(kt p) w x -> p kt (w x)", p=128))
                return q_t, k_t, v_t

            # pair-0 inputs first so the PE can start ASAP; constants go on
            # the ACT HWDGE ring so the two DMA streams run in parallel.
            prefetched = load_inputs(0)

            wq_sb = consts.tile([128, KT, C], bf16)
            nc.scalar.dma_start(wq_sb[:], wqt.rearrange("(kt p) o -> p kt o", p=128))
            wk_sb = consts.tile([128, KT, C], bf16)
            nc.scalar.dma_start(wk_sb[:], wkt.rearrange("(kt p) o -> p kt o", p=128))
            wv_sb = consts.tile([128, KT, C], bf16)
            nc.scalar.dma_start(wv_sb[:], wvt.rearrange("(kt p) o -> p kt o", p=128))
            wo_sb = consts.tile([128, KT, C], bf16)
            nc.scalar.dma_start(wo_sb[:], wot.rearrange("(kt p) o -> p kt o", p=128))
            qe8_sb = consts.tile([128, XT, H * QK], bf16)
            nc.scalar.dma_start(qe8_sb[:], qe8.rearrange("(xt p) m -> p xt m", p=128))
            ke_sb = consts.tile([128, XT, 2 * QK], bf16)
            nc.scalar.dma_start(ke_sb[:], ke2.rearrange("(xt p) m -> p xt m", p=128))
            ve_sb = consts.tile([128, X], f32)
            nc.scalar.dma_start(ve_sb[0:QK, :], vet[:])
            nc.scalar.dma_start(ve_sb[QK:128, :], vet[:])
            ones_sb = consts.tile([128, 2, 2], bf16)
            nc.scalar.dma_start(ones_sb[:], oned.rearrange("p (a b) -> p a b", a=2))

            for pair in range(PAIRS):
                w0 = pair * 2
                q_t, k_t, v_t = prefetched if pair == 0 else load_inputs(pair)

                # --- q/k projections, transposed layout: qsT/khT [x, (h c)] ---
                qsT = qkt.tile([128, 2, XT, C], bf16)   # [x_p, w, xt, o]
                khT = qkt.tile([128, 2, XT, C], bf16)
                khq = qkt.tile([128, 2, XT, C], bf16)   # khT + q_emb (folds t2)
                for wi in range(2):
                    for xt in range(XT):
                        pq = psA.tile([128, C], f32, tag="mm")
                        for kt in range(KT):
                            nc.tensor.matmul(
                                pq[:],
                                q_t[:, kt, wi, xt * 128:(xt + 1) * 128],
                                wq_sb[:, kt, :],
                                start=(kt == 0), stop=(kt == KT - 1))
                        nc.scalar.activation(qsT[:, wi, xt, 0:256], pq[:, 0:256],
                                             AF.Copy)
                        nc.vector.tensor_copy(qsT[:, wi, xt, 256:512],
                                              pq[:, 256:512])
                        pk = psA.tile([128, C], f32, tag="mm")
                        for kt in range(KT):
                            nc.tensor.matmul(
                                pk[:],
                                k_t[:, kt, wi, xt * 128:(xt + 1) * 128],
                                wk_sb[:, kt, :],
                                start=(kt == 0), stop=(kt == KT - 1))
                        nc.vector.tensor_copy(khT[:, wi, xt, 0:256], pk[:, 0:256])
                        nc.scalar.activation(khT[:, wi, xt, 256:512],
                                             pk[:, 256:512], AF.Copy)
                        nc.gpsimd.tensor_add(khq[:, wi, xt, :],
                                             khT[:, wi, xt, :], qe8_sb[:, xt, :])

                # --- v projection + ve add + ones column ---
                vplus = mid.tile([128, KT, 2, X + 2], bf16)  # [c2_p, head-pair, w, x+2]
                for ot in range(KT):
                    pv = psVL.tile([128, 2, X], f32, tag="vl")
                    for kt in range(KT):
                        nc.tensor.matmul(
                            pv[:],
                            wv_sb[:, kt, ot * 128:(ot + 1) * 128],
                            v_t[:, kt, :, :],
                            start=(kt == 0), stop=(kt == KT - 1))
                    for wi in range(2):
                        nc.vector.tensor_add(
                            vplus[:, ot, wi, 0:X], pv[:, wi, :], ve_sb[:])
                    nc.vector.tensor_copy(vplus[:, ot, :, X:X + 2], ones_sb[:])

                # --- per-w attention ---
                attn = mid.tile([128, KT, 2, X], bf16)  # [(h c)_p, kt, w, x]
                for wi in range(2):
                    pl = psVL.tile([128, C], f32, tag="vl")
                    # k_emb term, all heads at once (dup'd table)
                    nc.tensor.matmul(pl[:], ke_sb[:, 0, :], khT[:, wi, 0, :],
                                     start=True, stop=False)
                    nc.tensor.matmul(pl[:], ke_sb[:, 1, :], khT[:, wi, 1, :],
                                     start=False, stop=False)
                    # per-head (kh + qe)^T @ qs term (folds the q_emb term)
                    for h in range(H):
                        half = (h % 2) * QK
                        cb = h * QK
                        for xt in range(XT):
                            nc.tensor.matmul(
                                pl[half:half + QK, cb:cb + QK],
                                khq[:, wi, xt, cb:cb + QK],
                                qsT[:, wi, xt, cb:cb + QK],
                                start=False, stop=(h == H - 1 and xt == XT - 1),
                                tile_position=(0, half))
                    e_t = mid.tile([128, C], bf16, tag="e")
                    nc.scalar.activation(e_t[:], pl[:], AF.Exp)

                    for t in range(KT):          # head pairs (2t, 2t+1)
                        pu = psU.tile([128, X + 2], f32, tag="pu")
                        for j in range(2):       # j=0 even head, j=1 odd head
                            h = 2 * t + j
                            half = j * QK
                            nc.tensor.matmul(
                                pu[half:half + QK, :],
                                e_t[half:half + QK, h * QK:(h + 1) * QK],
                                vplus[half:half + QK, t, wi, :],
                                start=True, stop=True,
                                tile_position=(half, half))
                        recip = small.tile([128, 1], f32, tag="recip")
                        nc.vector.reciprocal(recip[:], pu[:, X:X + 1])
                        if t % 2 == 0:
                            nc.scalar.activation(
                                attn[:, t, wi, :],
                                pu[:, 0:X], AF.Copy, scale=recip[:])
                        else:
                            nc.vector.tensor_scalar_mul(
                                attn[:, t, wi, :], pu[:, 0:X], recip[:])

                # --- output projection ---
                for ot in range(KT):
                    po = psA.tile([128, 2, X], f32, tag="mm")
                    for kt in range(KT):
                        nc.tensor.matmul(
                            po[:],
                            wo_sb[:, kt, ot * 128:(ot + 1) * 128],
                            attn[:, kt, :, :],
                            start=(kt == 0), stop=(kt == KT - 1))
                    ob = mid.tile([128, 2, X], f32, tag="ob")
                    nc.scalar.activation(ob[:, 0, :], po[:, 0, :], AF.Copy)
                    nc.vector.tensor_copy(ob[:, 1, :], po[:, 1, :])
                    nc.sync.dma_start(
                        out[ot * 128:(ot + 1) * 128, w0:w0 + 2, :], ob[:])

    nc.compile()
    return nc


def _get_program():
    if "nc" not in _CACHE:
        _CACHE["nc"] = _build_program()
    return _CACHE["nc"]


def _make_in_maps(query, key_, value, Wq, Wk, Wv, Wo, q_emb, k_emb, v_emb):
    import ml_dtypes
    bf16 = ml_dtypes.bfloat16
    scale = np.float32(1.0 / np.sqrt(X))
    wqt = np.ascontiguousarray((Wq.T * scale).astype(bf16))
    wkt = np.ascontiguousarray(Wk.T.astype(bf16))
    wvt = np.ascontiguousarray(Wv.T.astype(bf16))
    wot = np.ascontiguousarray(Wo.T.astype(bf16))
    qe8 = np.ascontiguousarray(np.tile(q_emb, (1, H)).astype(bf16))
    ke2 = np.ascontiguousarray(np.concatenate([k_emb, k_emb], axis=1).astype(bf16))
    vet = np.ascontiguousarray(v_emb.T)
    def shard(a, ws):
        # (C, X, WC) -> [pair, i, w, x] contiguous, bf16
        return np.ascontiguousarray(
            a[:, :, ws].reshape(C, X, PAIRS, 2).transpose(2, 0, 3, 1).astype(bf16))

    in_maps = []
    for c in range(N_CORES):
        ws = slice(c * WC, (c + 1) * WC)
        in_maps.append({
            "qin": shard(query, ws),
            "kin": shard(key_, ws),
            "vin": shard(value, ws),
            "wqt": wqt, "wkt": wkt, "wvt": wvt, "wot": wot,
            "qe8": qe8, "ke2": ke2, "vet": vet,
            "oned": np.ones((128, 4), bf16),
        })
    return in_maps


def _run(in_maps, trace=False):
    from concourse.bass_utils import run_bass_kernel_spmd
    nc = _get_program()
    return run_bass_kernel_spmd(nc, in_maps, list(range(N_CORES)), trace=trace)


def kernel(query, key_, value, Wq, Wk, Wv, Wo, q_emb, k_emb, v_emb):
    args = (query, key_, value, Wq, Wk, Wv, Wo, q_emb, k_emb, v_emb)
    in_maps = _make_in_maps(*[np.ascontiguousarray(a, np.float32) for a in args])
    res = _run(in_maps, trace=False)
    out = np.empty((C, X, W), np.float32)
    for c in range(N_CORES):
        out[:, :, c * WC:(c + 1) * WC] = res.results[c]["out"].transpose(0, 2, 1)
    return out


# revision 22
# speedup vs baseline: 1.0415x; 1.0415x over previous
# Trainium2 Bass kernel for nn_AxialAttention (8 NeuronCores, head/W-parallel).
#
# Sharding: the W axis (axis=2, the vmapped axis) is split into 8 contiguous
# slices of 32 columns, one per core. Every part of the computation (the four
# 1x1-conv GEMMs, the per-(head, w) axial attention, the embedding terms) is
# independent across w, so there are no collectives; the small weight matrices
# and embedding tables are replicated to every core.
#
# Per-core math for one w column (all heads):
#   qsT[x, (h c)] = query[:, :, w].T @ (Wq.T / 16)    (scale folded into Wq)
#   khT[x, (h c)] = key_[:, :, w].T @ Wk.T
#   vh [(h c), x] = Wv @ value[:, :, w]
#   logits_h[C, c] = khT_h.T @ qsT_h + q_emb.T @ qsT_h + k_emb.T @ khT_h
#   E = exp(logits)             (max-subtraction unnecessary: |logits| < ~2)
#   U_h = E_h.T @ [vh_h + ve | 1]          (ones column gives the softmax
#   attn_h = U_h[:, :256] / U_h[:, 256]     denominator for free)
#   out[:, :, w] = Wo @ attn
#
# Heads are packed even/odd into the two 64-partition halves so the per-head
# 64x64 logits matmuls and the 64-row attention matmuls run as concurrent
# PE row/column tiles (tile_position diagonal packing). All matmuls run in
# bf16 with fp32 PSUM accumulation (measured 3.3e-3 absmax-relative error);
# large PSUM->SBUF evacuations are split across the Scalar and Vector engines
# to halve PSUM-slot release latency.

import numpy as np

H = 8          # heads
QK = 64        # per-head qk/vo channels
C = 512        # io channels
X = 256        # spatial H (attention contraction axis)
W = 256        # spatial W (vmapped axis, sharded)
N_CORES = 8
WC = W // N_CORES   # w columns per core
PAIRS = WC // 2

_CACHE = {}


def _build_program():
    import concourse.mybir as mybir
    import concourse.tile as tile
    from concourse import bacc

    f32 = mybir.dt.float32
    bf16 = mybir.dt.bfloat16
    AF = mybir.ActivationFunctionType

    nc = bacc.Bacc("TRN2", target_bir_lowering=False, debug=False,
                   num_devices=N_CORES)

    qin = nc.dram_tensor("qin", [PAIRS, C, 2, X], bf16, kind="ExternalInput").ap()
    kin = nc.dram_tensor("kin", [PAIRS, C, 2, X], bf16, kind="ExternalInput").ap()
    vin = nc.dram_tensor("vin", [PAIRS, C, 2, X], bf16, kind="ExternalInput").ap()
    wqt = nc.dram_tensor("wqt", [C, C], bf16, kind="ExternalInput").ap()
    wkt = nc.dram_tensor("wkt", [C, C], bf16, kind="ExternalInput").ap()
    wvt = nc.dram_tensor("wvt", [C, C], bf16, kind="ExternalInput").ap()
    wot = nc.dram_tensor("wot", [C, C], bf16, kind="ExternalInput").ap()
    qe8 = nc.dram_tensor("qe8", [X, H * QK], bf16, kind="ExternalInput").ap()
    ke2 = nc.dram_tensor("ke2", [X, 2 * QK], bf16, kind="ExternalInput").ap()
    vet = nc.dram_tensor("vet", [QK, X], f32, kind="ExternalInput").ap()
    oned = nc.dram_tensor("oned", [128, 4], bf16, kind="ExternalInput").ap()
    out = nc.dram_tensor("out", [C, WC, X], f32, kind="ExternalOutput").ap()

    KT = C // 128   # 4 contraction tiles of the channel dim
    XT = X // 128   # 2 tiles of the spatial-x dim

    with tile.TileContext(nc) as tc:
        with (
            tc.tile_pool(name="consts", bufs=1) as consts,
            tc.tile_pool(name="inp", bufs=3) as inp,
            tc.tile_pool(name="qkt", bufs=2) as qkt,
            tc.tile_pool(name="mid", bufs=2) as mid,
            tc.tile_pool(name="small", bufs=8) as small,
            tc.tile_pool(name="psA", bufs=3, space="PSUM") as psA,
            tc.tile_pool(name="psVL", bufs=2, space="PSUM") as psVL,
            tc.tile_pool(name="psU", bufs=3, space="PSUM") as psU,
        ):
            def load_inputs(pair):
                q_t = inp.tile([128, KT, 2, X], bf16, tag="q_t")
                nc.sync.dma_start(
                    q_t[:], qin[pair].rearrange("(kt p) w x -> p kt (w x)", p=128))
                k_t = inp.tile([128, KT, 2, X], bf16, tag="k_t")
                nc.sync.dma_start(
                    k_t[:], kin[pair].rearrange("(kt p) w x -> p kt (w x)", p=128))
                v_t = inp.tile([128, KT, 2, X], bf16, tag="v_t")
                nc.sync.dma_start(
                    v_t[:], vin[pair].rearrange("(kt p) w x -> p kt (w x)", p=128))
                return q_t, k_t, v_t

            # pair-0 inputs first so the PE can start ASAP; q is split per
            # k-tile so the first matmul only waits for one chunk. Constants
            # go on the ACT HWDGE ring so the two DMA streams run in parallel.
            q0 = inp.tile([128, KT, 2, X], bf16, tag="q_t")
            for kt in range(KT):
                nc.sync.dma_start(
                    q0[:, kt, :, :],
                    qin[0, kt * 128:(kt + 1) * 128].rearrange("p w x -> p (w x)"))
            k0 = inp.tile([128, KT, 2, X], bf16, tag="k_t")
            nc.sync.dma_start(
                k0[:], kin[0].rearrange("(kt p) w x -> p kt (w x)", p=128))
            v0 = inp.tile([128, KT, 2, X], bf16, tag="v_t")
            nc.sync.dma_start(
                v0[:], vin[0].rearrange("(kt p) w x -> p kt (w x)", p=128))
            prefetched = (q0, k0, v0)

            wq_sb = consts.tile([128, KT, C], bf16)
            nc.scalar.dma_start(wq_sb[:], wqt.rearrange("(kt p) o -> p kt o", p=128))
            wk_sb = consts.tile([128, KT, C], bf16)
            nc.scalar.dma_start(wk_sb[:], wkt.rearrange("(kt p) o -> p kt o", p=128))
            wv_sb = consts.tile([128, KT, C], bf16)
            nc.scalar.dma_start(wv_sb[:], wvt.rearrange("(kt p) o -> p kt o", p=128))
            wo_sb = consts.tile([128, KT, C], bf16)
            nc.scalar.dma_start(wo_sb[:], wot.rearrange("(kt p) o -> p kt o", p=128))
            qe8_sb = consts.tile([128, XT, H * QK], bf16)
            nc.scalar.dma_start(qe8_sb[:], qe8.rearrange("(xt p) m -> p xt m", p=128))
            ke_sb = consts.tile([128, XT, 2 * QK], bf16)
            nc.scalar.dma_start(ke_sb[:], ke2.rearrange("(xt p) m -> p xt m", p=128))
            ve_sb = consts.tile([128, X], f32)
            nc.scalar.dma_start(ve_sb[0:QK, :], vet[:])
            nc.scalar.dma_start(ve_sb[QK:128, :], vet[:])
            ones_sb = consts.tile([128, 2, 2], bf16)
            nc.scalar.dma_start(ones_sb[:], oned.rearrange("p (a b) -> p a b", a=2))

            for pair in range(PAIRS):
                w0 = pair * 2
                q_t, k_t, v_t = prefetched if pair == 0 else load_inputs(pair)

                # --- q/k projections, transposed layout: qsT/khT [x, (h c)] ---
                qsT = qkt.tile([128, 2, XT, C], bf16)   # [x_p, w, xt, o]
                khT = qkt.tile([128, 2, XT, C], bf16)
                khq = qkt.tile([128, 2, XT, C], bf16)   # khT + q_emb (folds t2)
                for wi in range(2):
                    for xt in range(XT):
                        pq = psA.tile([128, C], f32, tag="mm")
                        for kt in range(KT):
                            nc.tensor.matmul(
                                pq[:],
                                q_t[:, kt, wi, xt * 128:(xt + 1) * 128],
                                wq_sb[:, kt, :],
                                start=(kt == 0), stop=(kt == KT - 1))
                        nc.scalar.activation(qsT[:, wi, xt, 0:256], pq[:, 0:256],
                                             AF.Copy)
                        nc.vector.tensor_copy(qsT[:, wi, xt, 256:512],
                                              pq[:, 256:512])
                        pk = psA.tile([128, C], f32, tag="mm")
                        for kt in range(KT):
                            nc.tensor.matmul(
                                pk[:],
                                k_t[:, kt, wi, xt * 128:(xt + 1) * 128],
                                wk_sb[:, kt, :],
                                start=(kt == 0), stop=(kt == KT - 1))
                        nc.vector.tensor_copy(khT[:, wi, xt, 0:256], pk[:, 0:256])
                        nc.scalar.activation(khT[:, wi, xt, 256:512],
                                             pk[:, 256:512], AF.Copy)
                        nc.gpsimd.tensor_add(khq[:, wi, xt, :],
                                             khT[:, wi, xt, :], qe8_sb[:, xt, :])

                # --- v projection + ve add + ones column ---
                vplus = mid.tile([128, KT, 2, X + 2], bf16)  # [c2_p, head-pair, w, x+2]
                for ot in range(KT):
                    pv = psVL.tile([128, 2, X], f32, tag="vl")
                    for kt in range(KT):
                        nc.tensor.matmul(
                            pv[:],
                            wv_sb[:, kt, ot * 128:(ot + 1) * 128],
                            v_t[:, kt, :, :],
                            start=(kt == 0), stop=(kt == KT - 1))
                    for wi in range(2):
                        nc.vector.tensor_add(
                            vplus[:, ot, wi, 0:X], pv[:, wi, :], ve_sb[:])
                    nc.vector.tensor_copy(vplus[:, ot, :, X:X + 2], ones_sb[:])

                # --- per-w attention ---
                attn = mid.tile([128, KT, 2, X], bf16)  # [(h c)_p, kt, w, x]
                for wi in range(2):
                    pl = psVL.tile([128, C], f32, tag="vl")
                    # k_emb term, all heads at once (dup'd table)
                    nc.tensor.matmul(pl[:], ke_sb[:, 0, :], khT[:, wi, 0, :],
                                     start=True, stop=False)
                    nc.tensor.matmul(pl[:], ke_sb[:, 1, :], khT[:, wi, 1, :],
                                     start=False, stop=False)
                    # per-head (kh + qe)^T @ qs term (folds the q_emb term)
                    for h in range(H):
                        half = (h % 2) * QK
                        cb = h * QK
                        for xt in range(XT):
                            nc.tensor.matmul(
                                pl[half:half + QK, cb:cb + QK],
                                khq[:, wi, xt, cb:cb + QK],
                                qsT[:, wi, xt, cb:cb + QK],
                                start=False, stop=(h == H - 1 and xt == XT - 1),
                                tile_position=(0, half))
                    e_t = mid.tile([128, C], bf16, tag="e")
                    nc.scalar.activation(e_t[:], pl[:], AF.Exp)

                    for t in range(KT):          # head pairs (2t, 2t+1)
                        pu = psU.tile([128, X + 2], f32, tag="pu")
                        for j in range(2):       # j=0 even head, j=1 odd head
                            h = 2 * t + j
                            half = j * QK
                            nc.tensor.matmul(
                                pu[half:half + QK, :],
                                e_t[half:half + QK, h * QK:(h + 1) * QK],
                                vplus[half:half + QK, t, wi, :],
                                start=True, stop=True,
                                tile_position=(half, half))
                        recip = small.tile([128, 1], f32, tag="recip")
                        nc.vector.reciprocal(recip[:], pu[:, X:X + 1])
                        if t % 2 == 0:
                            nc.scalar.activation(
                                attn[:, t, wi, :],
                                pu[:, 0:X], AF.Copy, scale=recip[:])
                        else:
                            nc.vector.tensor_scalar_mul(
                                attn[:, t, wi, :], pu[:, 0:X], recip[:])

                # --- output projection ---
                for ot in range(KT):
                    po = psVL.tile([128, 2, X], f32, tag="vl")
                    for kt in range(KT):
                        nc.tensor.matmul(
                            po[:],
                            wo_sb[:, kt, ot * 128:(ot + 1) * 128],
                            attn[:, kt, :, :],
                            start=(kt == 0), stop=(kt == KT - 1))
                    ob = mid.tile([128, 2, X], f32, tag="ob")
                    nc.scalar.activation(ob[:, 0, :], po[:, 0, :], AF.Copy)
                    nc.vector.tensor_copy(ob[:, 1, :], po[:, 1, :])
                    nc.sync.dma_start(
                        out[ot * 128:(ot + 1) * 128, w0:w0 + 2, :], ob[:])

    nc.compile()
    return nc


def _get_program():
    if "nc" not in _CACHE:
        _CACHE["nc"] = _build_program()
    return _CACHE["nc"]


def _make_in_maps(query, key_, value, Wq, Wk, Wv, Wo, q_emb, k_emb, v_emb):
    import ml_dtypes
    bf16 = ml_dtypes.bfloat16
    scale = np.float32(1.0 / np.sqrt(X))
    wqt = np.ascontiguousarray((Wq.T * scale).astype(bf16))
    wkt = np.ascontiguousarray(Wk.T.astype(bf16))
    wvt = np.ascontiguousarray(Wv.T.astype(bf16))
    wot = np.ascontiguousarray(Wo.T.astype(bf16))
    qe8 = np.ascontiguousarray(np.tile(q_emb, (1, H)).astype(bf16))
    ke2 = np.ascontiguousarray(np.concatenate([k_emb, k_emb], axis=1).astype(bf16))
    vet = np.ascontiguousarray(v_emb.T)
    def shard(a, ws):
        # (C, X, WC) -> [pair, i, w, x] contiguous, bf16
        return np.ascontiguousarray(
            a[:, :, ws].reshape(C, X, PAIRS, 2).transpose(2, 0, 3, 1).astype(bf16))

    in_maps = []
    for c in range(N_CORES):
        ws = slice(c * WC, (c + 1) * WC)
        in_maps.append({
            "qin": shard(query, ws),
            "kin": shard(key_, ws),
            "vin": shard(value, ws),
            "wqt": wqt, "wkt": wkt, "wvt": wvt, "wot": wot,
            "qe8": qe8, "ke2": ke2, "vet": vet,
            "oned": np.ones((128, 4), bf16),
        })
    return in_maps


def _run(in_maps, trace=False):
    from concourse.bass_utils import run_bass_kernel_spmd
    nc = _get_program()
    return run_bass_kernel_spmd(nc, in_maps, list(range(N_CORES)), trace=trace)


def kernel(query, key_, value, Wq, Wk, Wv, Wo, q_emb, k_emb, v_emb):
    args = (query, key_, value, Wq, Wk, Wv, Wo, q_emb, k_emb, v_emb)
    in_maps = _make_in_maps(*[np.ascontiguousarray(a, np.float32) for a in args])
    res = _run(in_maps, trace=False)
    out = np.empty((C, X, W), np.float32)
    for c in range(N_CORES):
        out[:, :, c * WC:(c + 1) * WC] = res.results[c]["out"].transpose(0, 2, 1)
    return out
